# revision 36
# baseline (speedup 1.0000x reference)
"""Trainium2 Bass kernel for nn_ANO_VQC_Model (14-qubit VQC, batch 512).

Math: the circuit state, viewed as a 128x128 matrix M (rows = qubits 0-6,
cols = qubits 7-13), starts as a real rank-1 outer product u v^T and each
entangling layer k acts as M' = A_k CNOT67(M) B_k^T with A_k, B_k pure
orthogonal local operators and CNOT67(M) = E0 M + E1 M F (E0/E1 = projectors
on qubit 6 = row LSB, F = X on qubit 7 = column-half swap).  So the state
stays factored L R^T with L <- A[E0 L | E1 L], R <- B[R | F R]; rank doubles
per layer.

Only 5 layers are simulated (rank 32).  The 6th layer is folded into the
measurement: with M6 = A6 N B6^T, N = CNOT67(M5), orthogonality of B6 gives
    e_q = <T_dq, S> + <T_oq, S'>,
    T_dq = L^T Hd_q L, T_oq = L^T Ho_q L, S = R^T R, S' = R^T F R,
where Hd_q/Ho_q are the qubit-6 block-diagonal/off-diagonal parts of
A6^T Re(H_q) A6.  A host-side orthogonal rotation Q diagonalizes Hd_0
(folded into the stage-2 A matrices), so T_d0 needs only a per-partition
scale of L.

Device schedule (v3): stage1 = layers 0-1 (b-major psum writes so stage2
streams contiguously), stage2 = layers 2-4 (contiguous 256-col rhs), K =
the three measurement matrix products (512-col psum chunks), per-sample
Grams (4-sample PE column tiling, ~34ns/mm issue rate), psum-direct
multiplies + c-accumulated selection matmuls + PSUM->SBUF reduces.  All
matmuls fp16 (fp8 was measured: each state quantization costs ~2.5e-2 rel
err, over budget; stride-0 DoubleRow gives no speedup).  Weights arrive as
packed tensors (1-2KB DMA rows) and the warmup is reduced to two bridge
matmuls so real work starts immediately.

Sharding: pure data parallel, 64 batch elements per core on 8 cores.
"""

import os
import sys

import numpy as np

for _p in ("/opt/trn_rl_repo", "/root/.axon_site/_ro/trn_rl_repo"):
    if os.path.isdir(_p) and _p not in sys.path:
        sys.path.append(_p)

import concourse.bass as bass
import concourse.mybir as mybir
import concourse.tile as tile
from concourse import bacc
from concourse.bass_utils import run_bass_kernel_spmd


def _ensure_ntff_hook():
    """bass_utils imports antenv.axon_hooks when tracing; some images lack
    it.  Provide a shim (and register the ctypes NTFF hook when possible)."""
    try:
        import antenv.axon_hooks  # noqa: F401

        return
    except ImportError:
        pass
    try:
        import types

        import antenv

        mod = types.ModuleType("antenv.axon_hooks")
        holder = {}
        mod.set_axon_ntff_profile_hook = lambda h: holder.__setitem__("h", h)
        mod.get_axon_ntff_profile_hook = lambda: holder.get("h")
        sys.modules["antenv.axon_hooks"] = mod
        antenv.axon_hooks = mod
        try:
            from trn_agent_boot.trn_boot import _ntff_profile_via_ctypes

            hook = _ntff_profile_via_ctypes("/opt/axon/libaxon_pjrt.so")
            if hook is not None:
                mod.set_axon_ntff_profile_hook(hook)
        except Exception:
            pass
    except Exception:
        pass


_ensure_ntff_hook()

N_CORES = 8
BATCH = 512
BPC = BATCH // N_CORES  # 64
DEPTH = 6
DA = 128
DB = 128

F32 = mybir.dt.float32
F16 = mybir.dt.float16
F8 = mybir.dt.float8e4
DRMODE = mybir.MatmulPerfMode.DoubleRow

USE_FP8 = os.environ.get("VQC_FP8", "0") == "1"
MM8 = F8 if USE_FP8 else F16

_nc_cache = {}


# ----------------------------------------------------------------------------
# Host-side preprocessing (input-dependent constant folding)
# ----------------------------------------------------------------------------

def _ry(t):
    c, s = np.cos(t / 2), np.sin(t / 2)
    return np.array([[c, -s], [s, c]], dtype=np.float64)


_CNOT = np.array(
    [[1, 0, 0, 0], [0, 1, 0, 0], [0, 0, 0, 1], [0, 0, 1, 0]], dtype=np.float64
)


def _kron_list(ms):
    out = ms[0]
    for m in ms[1:]:
        out = np.kron(out, m)
    return out


def _cnot_on(n, ctrl):
    mats, q = [], 0
    while q < n:
        if q == ctrl:
            mats.append(_CNOT)
            q += 2
        else:
            mats.append(np.eye(2))
            q += 1
    return _kron_list(mats)


def _layer_ops(theta_k):
    """Pure-orthogonal (A, B) for one layer; CNOT67 handled separately."""
    C_evenA = _cnot_on(7, 0) @ _cnot_on(7, 2) @ _cnot_on(7, 4)
    C_oddA = _cnot_on(7, 1) @ _cnot_on(7, 3) @ _cnot_on(7, 5)
    R_A = _kron_list([_ry(theta_k[w]) for w in range(7)])
    C_evenB = _cnot_on(7, 1) @ _cnot_on(7, 3) @ _cnot_on(7, 5)
    C_oddB = _cnot_on(7, 0) @ _cnot_on(7, 2) @ _cnot_on(7, 4)
    R_B = _kron_list([_ry(theta_k[7 + w]) for w in range(7)])
    return R_A @ C_oddA @ C_evenA, R_B @ C_oddB @ C_evenB


def _measure_mats(Ain, Din):
    """G_q = Re(H_q) expanded on the 128-dim row space, q = 0, 1."""
    NLOC = 8
    r, c = np.tril_indices(NLOC, -1)
    Gs = []
    for q in range(2):
        tri = np.zeros((NLOC, NLOC))
        tri[r, c] = Ain[q]
        h = tri + np.diag(np.concatenate([Din[q][1:], [0.0]]))
        Hr = h + h.T
        if q == 0:
            Gs.append(np.kron(Hr, np.eye(16)))
        else:
            Gs.append(np.kron(np.kron(np.eye(2), Hr), np.eye(8)))
    return Gs


def _host_prep(X, theta, Ain, Bin, Din):
    X = np.asarray(X, dtype=np.float64)
    theta = np.asarray(theta, dtype=np.float64)
    nb = X.shape[0]
    c, s = np.cos(X / 2), np.sin(X / 2)
    v0 = (c - s) / np.sqrt(2.0)
    v1 = (c + s) / np.sqrt(2.0)

    def kron_side(ws):
        out = np.ones((nb, 1))
        for w in ws:
            pair = np.stack([v0[:, w], v1[:, w]], axis=1)
            out = (out[:, :, None] * pair[:, None, :]).reshape(nb, -1)
        return out

    U = kron_side(range(7))       # (B, 128), qubit 0 = MSB ... qubit 6 = LSB
    V = kron_side(range(7, 14))   # (B, 128), qubit 7 = MSB

    As, Bs = zip(*[_layer_ops(theta[k]) for k in range(DEPTH)])
    rows = np.arange(DA)
    e0 = (rows % 2 == 0).astype(np.float64)
    E = [np.diag(e0), np.diag(1.0 - e0)]
    F = np.zeros((DB, DB))
    F[:64, 64:] = np.eye(64)
    F[64:, :64] = np.eye(64)

    # measurement: fold layer 6, rotate rows by Q diagonalizing Hd_0
    G0, G1 = _measure_mats(Ain, Din)
    A6 = As[5]
    Hds, Hos = [], []
    for G in (G0, G1):
        Ht = A6.T @ G @ A6
        Hds.append(E[0] @ Ht @ E[0] + E[1] @ Ht @ E[1])
        Hos.append(E[0] @ Ht @ E[1] + E[1] @ Ht @ E[0])
    mu, Q = np.linalg.eigh(Hds[0])
    hk = np.stack(
        [Q.T @ Hos[0] @ Q, Q.T @ Hds[1] @ Q, Q.T @ Hos[1] @ Q], axis=0
    )  # (3, 128, 128) symmetric

    # stage 1: layers 0-1, 4 terms; c2 = p1*2 + p0
    F1A = np.empty((4, DA, DA))
    F1B = np.empty((4, DB, DB))
    for cw in range(4):
        p0, p1 = cw & 1, (cw >> 1) & 1
        F1A[cw] = As[1] @ E[p1] @ As[0] @ E[p0]
        F1B[cw] = (
            Bs[1] @ np.linalg.matrix_power(F, p1)
            @ Bs[0] @ np.linalg.matrix_power(F, p0)
        )
    # stage 2: layers 2-4, 8 terms; a = p4*4 + p3*2 + p2; Q^T folded into A
    F2A = np.empty((8, DA, DA))
    F2B = np.empty((8, DB, DB))
    for aw_ in range(8):
        p2, p3, p4 = aw_ & 1, (aw_ >> 1) & 1, (aw_ >> 2) & 1
        F2A[aw_] = Q.T @ As[4] @ E[p4] @ As[3] @ E[p3] @ As[2] @ E[p2]
        F2B[aw_] = (
            Bs[4] @ np.linalg.matrix_power(F, p4)
            @ Bs[3] @ np.linalg.matrix_power(F, p3)
            @ Bs[2] @ np.linalg.matrix_power(F, p2)
        )

    # lhsT packs.  B-side F-pairing: F1B[(p1,1)] = F1B[(p1,0)] @ F, so store
    # only p0=0 terms and stream [vt | F vt]; the F-premultiplied copies
    # (F @ F1B, F @ F2B) produce the partition-swapped states F R directly,
    # so no SBUF-SBUF swap DMAs are needed anywhere.
    wa1 = np.concatenate([F1A[cw].T for cw in range(4)], axis=1)  # (128, 512)
    wa2 = np.concatenate([F2A[aw_].T for aw_ in range(8)], axis=1)  # (128,1024)
    wb1c = np.concatenate(
        [F1B[p1 * 2].T for p1 in range(2)]
        + [(F @ F1B[p1 * 2]).T for p1 in range(2)],
        axis=1,
    )  # (128, 512): [normal(2) | F-pre(2)]; term (p1,p0) uses rhs half p0
    wb2c = np.concatenate(
        [F2B[(p4 * 2 + p3) * 2].T for p4 in range(2) for p3 in range(2)],
        axis=1,
    )  # (128, 512), ab = p4*2+p3
    wb2f = np.concatenate(
        [(F @ F2B[(p4 * 2 + p3) * 2]).T for p4 in range(2) for p3 in range(2)],
        axis=1,
    )  # (128, 512)
    whk = np.concatenate([hk[i] for i in range(3)], axis=1)  # (128, 384)
    return U, V, wa1, wb1c, wa2, wb2c, wb2f, whk, mu


# ----------------------------------------------------------------------------
# Device kernel
# ----------------------------------------------------------------------------

def _dr(ap, n):
    """Duplicate the contraction as a stride-0 second k-subtile (DoubleRow)."""
    return ap.unsqueeze(1).broadcast_to((128, 2) + tuple(n))


def _build_nc():
    nc = bacc.Bacc("TRN2", target_bir_lowering=False, debug=False)

    # pk1 = [ut | wa1c], pk2 = [vt | F vt | wb1c]; parity-compressed A-side
    # weights, F-paired B-side weights (half the DMA bytes each)
    pk1_d = nc.declare_dram_parameter("pk1", [DA, 576], F16, isOutput=False)
    pk2_d = nc.declare_dram_parameter("pk2", [DB, 640], F16, isOutput=False)
    wa2_d = nc.declare_dram_parameter("wa2c", [DA, 1024], F16, isOutput=False)
    wb2_d = nc.declare_dram_parameter("wb2c", [DB, 512], F16, isOutput=False)
    wb2f_d = nc.declare_dram_parameter("wb2f", [DB, 512], F16, isOutput=False)
    whk_d = nc.declare_dram_parameter("whk", [DA, 384], F16, isOutput=False)
    mu_d = nc.declare_dram_parameter("mu", [DA, 1], F32, isOutput=False)
    out_d = nc.declare_dram_parameter("out", [4, 32], F32, isOutput=True)

    with tile.TileContext(nc) as tc:
        with (
            tc.tile_pool(name="w", bufs=1) as wpool,
            tc.tile_pool(name="state", bufs=1) as spool,
            tc.tile_pool(name="ps", bufs=3, space="PSUM") as pbig,   # 2 banks each
            tc.tile_pool(name="ps2", bufs=2, space="PSUM") as psmall,  # 1 bank each
        ):
            pk1 = wpool.tile([DA, 576], F16, tag="pk1")
            pk2 = wpool.tile([DB, 640], F16, tag="pk2")
            wa2 = wpool.tile([DA, 1024], F16, tag="wa2c")
            wb2 = wpool.tile([DB, 512], F16, tag="wb2c")
            wb2f = wpool.tile([DB, 512], F16, tag="wb2f")
            hkw = wpool.tile([DA, 384], F16, tag="hkw")
            muT = wpool.tile([DA, 1], F32, tag="mu")
            sel = wpool.tile([128, 4], F16, tag="sel")
            warm = wpool.tile([128, 512], F16, tag="warm")

            ut = pk1[:, 0:64]
            wa1 = pk1[:, 64:576]
            vtp = pk2[:, 0:128]
            wb1 = pk2[:, 128:640]

            # input DMAs over the three DMA-capable queues (~48, ~48,
            # ~32 GB/s observed), earliest-needed first per queue
            nc.sync.dma_start(out=pk1[:], in_=pk1_d[:, :])
            nc.scalar.dma_start(out=wa2[:], in_=wa2_d[:, :])
            nc.gpsimd.dma_start(out=pk2[:], in_=pk2_d[:, :])
            nc.sync.dma_start(out=hkw[:], in_=whk_d[:, :])
            nc.scalar.dma_start(out=wb2f[:], in_=wb2f_d[:, :])
            nc.gpsimd.dma_start(out=wb2[:], in_=wb2_d[:, :])
            nc.gpsimd.dma_start(out=muT[:], in_=mu_d[:, :])

            nc.vector.memset(warm[:], 0.125)
            nc.vector.memset(sel[:], 0.0)
            for m in range(4):
                nc.vector.memset(sel[32 * m:32 * m + 32, m:m + 1], 1.0)

            # one warmup matmul bridges the PE until the first input DMA lands
            wps = pbig.tile([128, 1024], F32, tag="big")
            nc.tensor.matmul(
                wps[:, 0:512], warm[:, 0:128], warm[:], start=True, stop=True
            )
            nc.tensor.matmul(
                wps[:, 512:1024], warm[:, 0:128], warm[:], start=True, stop=True
            )

            L3 = spool.tile([DA, 256], MM8, tag="L3")      # cols (b, c)
            R3p = spool.tile([DB, 512], MM8, tag="R3p")    # [R3 | F R3]
            Lb = spool.tile([DA, 2048], MM8, tag="Lb")     # cols (b, a, c)
            RF = spool.tile([DB, 4096], MM8, tag="RF")     # 0:2048 R, 2048: FR
            Pb = spool.tile([DA, 8192], MM8, tag="Pb")     # cols (b, s4, i32)
            SS = spool.tile([128, 1024], F16, tag="SS")
            tb = spool.tile([128, 2048], F16, tag="tb")
            esb = spool.tile([4, 32], F32, tag="esb")

            # ---- stage 1: 4 terms per side, b-major psum writes -------------
            # A side: parity-compressed weights, 64-deep contraction on the
            # partition half selected by p0.  B side: F-paired weights with
            # the [vt | F vt] rhs pack.  Separate psum tiles per side so the
            # L3 evict does not wait on the B-side matmuls.
            s1t = psmall.tile([128, 256], F32, tag="small")
            s1u = psmall.tile([128, 512], F32, tag="small")
            s1a = s1t[:].rearrange("p (b c) -> p b c", c=4)
            s1b = s1u[:].rearrange("p (f b c) -> p f b c", f=2, c=4)
            for cw in range(4):
                nc.tensor.matmul(
                    s1a[:, :, cw],
                    wa1[:, cw * 128:(cw + 1) * 128], ut,
                    start=True, stop=True, skip_group_check=True,
                )
            nc.vector.tensor_copy(L3[:], s1t[:])

            # gap fillers: dependency-free matmuls keep the PE continuously
            # busy while the wa2 DMA lands, sustaining the p-state ramp so
            # stage 2 runs at full clock
            for _ in range(3):
                nc.tensor.matmul(
                    wps[:, 0:512], warm[:, 0:128], warm[:],
                    start=True, stop=True,
                )

            # ---- stage 2 A first (only needs L3/wa2): keeps the in-order PE
            # queue from stalling on the later pk2 DMA ----------------------
            Lbv = Lb[:].rearrange("p (b a c) -> p a b c", a=8, c=4)
            Rbv = RF[:, 0:2048].rearrange("p (b a c) -> p a b c", a=8, c=4)
            evict_engines = [nc.vector, nc.scalar]

            def coarse_evict(eng, dst, srcv):
                if eng is nc.scalar:
                    eng.copy(out=dst, in_=srcv)
                else:
                    eng.tensor_copy(dst, srcv)

            # A side: 8 terms, contiguous 256-col rhs
            for half in range(2):
                s2 = pbig.tile([128, 1024], F32, tag="big")
                for i in range(4):
                    a = 4 * half + i
                    nc.tensor.matmul(
                        s2[:, i * 256:(i + 1) * 256],
                        wa2[:, a * 128:(a + 1) * 128], L3[:],
                        start=True, stop=True,
                    )
                s2v = s2[:].rearrange("p (i b c) -> p b i c", i=4, c=4)
                coarse_evict(
                    evict_engines[half % 2],
                    Lbv[:, 4 * half:4 * half + 4].rearrange(
                        "p a b c -> p b a c"
                    ),
                    s2v,
                )

            Pv0 = Pb[:].rearrange("p (b s i) -> p s b i", s=4, i=32)

            # ---- stage 1 B (waits on the pk2 DMA) ---------------------------
            for fw in range(2):
                for cw in range(4):
                    p0, p1 = cw & 1, (cw >> 1) & 1
                    nc.tensor.matmul(
                        s1b[:, fw, :, cw],
                        wb1[:, (fw * 2 + p1) * 128:(fw * 2 + p1 + 1) * 128],
                        vtp[:, p0 * 64:(p0 + 1) * 64],
                        start=True, stop=True, skip_group_check=True,
                    )
            nc.scalar.copy(out=R3p[:], in_=s1u[:])
            # B side: ab = p4*2 + p3; one matmul streams [R3 | F R3] and
            # produces both p2 terms (a = 2*ab + p2).  The wb2f pass emits
            # the partition-swapped F R states straight into RF[:, 2048:].
            FRbv = RF[:, 2048:4096].rearrange("p (b a c) -> p a b c", a=8, c=4)
            for fw, (wtile, dstv) in enumerate(((wb2, Rbv), (wb2f, FRbv))):
                for half in range(2):
                    s2 = pbig.tile([128, 1024], F32, tag="big")
                    for m in range(2):
                        ab = 2 * half + m
                        nc.tensor.matmul(
                            s2[:, m * 512:(m + 1) * 512],
                            wtile[:, ab * 128:(ab + 1) * 128],
                            R3p[:],
                            start=True, stop=True,
                        )
                    s2v = s2[:].rearrange(
                        "p (m f b c) -> p b m f c", m=2, f=2, c=4
                    )
                    coarse_evict(
                        evict_engines[(half + fw + 1) % 2],
                        dstv[:, 4 * half:4 * half + 4].rearrange(
                            "p (m f) b c -> p b m f c", m=2
                        ),
                        s2v,
                    )

            # ---- K matmuls: 256-col chunks into 1-bank psum tiles -----------
            # rhs chunk = Lb[b-half, a-pair j] -> [p, 32, 8] contiguous runs
            Lbk = Lb[:].rearrange("p (b i) -> p b i", i=32)
            Pbv = Pb[:].rearrange("p (b s i) -> p b s i", s=4, i=32)

            def k_mat(im):
                for bh in range(2):
                    kp = pbig.tile([128, 1024], F32, tag="big")
                    for j in range(4):
                        rhs = Lbk[:, 32 * bh:32 * bh + 32, 8 * j:8 * j + 8]
                        lhs = hkw[:, im * 128:(im + 1) * 128]
                        nc.tensor.matmul(
                            kp[:, j * 256:(j + 1) * 256], lhs, rhs,
                            start=True, stop=True,
                        )
                    # evict: psum (j, b32, i8) -> Pb[b, im+1, (j,i8)]
                    kv = kp[:].rearrange("p (j b i) -> p b j i", j=4, i=8)
                    dst = Pbv[:, 32 * bh:32 * bh + 32, im + 1].rearrange(
                        "p b (j i) -> p b j i", j=4
                    )
                    eng = evict_engines[(im + bh) % 2]
                    if eng is nc.scalar:
                        eng.copy(out=dst, in_=kv)
                    else:
                        eng.tensor_copy(dst, kv)

            # ---- per-sample S-Grams: rhs = [R_b | FR_b] ---------------------
            RFs = RF[:].rearrange("p (h b i) -> p b h i", h=2, i=32)

            def gram_s(g2):
                sp = psmall.tile([128, 512], F32, tag="small")
                for g in range(8 * g2, 8 * g2 + 8):
                    for k in range(4):
                        b = g * 4 + k
                        nc.tensor.matmul(
                            sp[32 * k:32 * k + 32,
                               (g - 8 * g2) * 64:(g - 8 * g2 + 1) * 64],
                            RF[:, b * 32:(b + 1) * 32],
                            RFs[:, b],
                            start=True, stop=True, tile_position=(0, 32 * k),
                        )
                nc.scalar.copy(out=SS[:, g2 * 512:(g2 + 1) * 512], in_=sp[:])

            # interleave K-matrix matmuls with S-Grams to keep the PE dense;
            # mu*L (ACT, per-partition scale) sits after the early k-evicts
            k_mat(0)
            k_mat(1)
            nc.vector.tensor_scalar_mul(
                Pv0[:, 0], Lb[:].rearrange("p (b i) -> p b i", i=32), muT[:]
            )
            gram_s(0)
            k_mat(2)
            gram_s(1)

            # ---- T-Grams + psum-direct multiply -----------------------------
            for g2 in range(2):
                tp = pbig.tile([128, 1024], F32, tag="big")
                for g in range(8 * g2, 8 * g2 + 8):
                    for k in range(4):
                        b = g * 4 + k
                        nc.tensor.matmul(
                            tp[32 * k:32 * k + 32,
                               (g - 8 * g2) * 128:(g - 8 * g2 + 1) * 128],
                            Lb[:, b * 32:(b + 1) * 32],
                            Pb[:, b * 128:(b + 1) * 128],
                            start=True, stop=True, tile_position=(0, 32 * k),
                        )
                ssv = SS[:, g2 * 512:(g2 + 1) * 512].rearrange(
                    "p (g j) -> p g j", g=8
                ).unsqueeze(2).broadcast_to((128, 8, 2, 64))
                tbv = tb[:, g2 * 1024:(g2 + 1) * 1024].rearrange(
                    "p (g q j) -> p g q j", g=8, q=2, j=64
                )
                tpv = tp[:].rearrange("p (g q j) -> p g q j", g=8, q=2, j=64)
                # psum-direct multiply on DVE (two chunks for pipelining)
                nc.vector.tensor_mul(tbv[:, 0:4], tpv[:, 0:4], ssv[:, 0:4])
                nc.vector.tensor_mul(tbv[:, 4:8], tpv[:, 4:8], ssv[:, 4:8])

            # ---- partition reduce: c-accumulated selection matmuls ----------
            zp0 = psmall.tile([4, 256], F32, tag="small")
            zp1 = psmall.tile([4, 256], F32, tag="small")
            zpq = [zp0, zp1]
            for g2 in range(2):
                tbq = tb[:, g2 * 1024:(g2 + 1) * 1024].rearrange(
                    "p (g q c j) -> p q c g j", g=8, q=2, c=4, j=16
                )
                for q in range(2):
                    for c in range(4):
                        nc.tensor.matmul(
                            zpq[q][:, g2 * 128:(g2 + 1) * 128],
                            sel[:], tbq[:, q, c],
                            start=(c == 0), stop=(c == 3),
                        )
            for q in range(2):
                nc.vector.reduce_sum(
                    out=esb[:, q * 16:(q + 1) * 16],
                    in_=zpq[q][:].rearrange("p (g j) -> p g j", j=16),
                    axis=mybir.AxisListType.X,
                )
            nc.sync.dma_start(out=out_d[:, :], in_=esb[:])

    nc.compile()
    return nc


def _get_nc():
    if "nc" not in _nc_cache:
        _nc_cache["nc"] = _build_nc()
    return _nc_cache["nc"]


# ----------------------------------------------------------------------------
# Entry point
# ----------------------------------------------------------------------------

def _to_mm8(x):
    if USE_FP8:
        import ml_dtypes

        return np.ascontiguousarray(x, dtype=ml_dtypes.float8_e4m3fn)
    return np.ascontiguousarray(x, dtype=np.float16)


def kernel(X, theta, A, B, D, _trace=False):
    U, V, wa1, wb1c, wa2, wb2c, wb2f, whk, mu = _host_prep(X, theta, A, B, D)
    wa2_a = np.ascontiguousarray(wa2, dtype=np.float16)
    wb2_a = np.ascontiguousarray(wb2c, dtype=np.float16)
    wb2f_a = np.ascontiguousarray(wb2f, dtype=np.float16)
    whk_a = np.ascontiguousarray(whk, dtype=np.float16)
    mu_a = np.ascontiguousarray(mu.reshape(DA, 1), dtype=np.float32)
    in_maps = []
    for i in range(N_CORES):
        sl = slice(i * BPC, (i + 1) * BPC)
        vt = V[sl].T
        fvt = np.concatenate([vt[64:128], vt[0:64]], axis=0)
        pk1 = np.ascontiguousarray(
            np.concatenate([U[sl].T, wa1], axis=1), dtype=np.float16
        )
        pk2 = np.ascontiguousarray(
            np.concatenate([vt, fvt, wb1c], axis=1), dtype=np.float16
        )
        in_maps.append(
            {"pk1": pk1, "pk2": pk2, "wa2c": wa2_a, "wb2c": wb2_a,
             "wb2f": wb2f_a, "whk": whk_a, "mu": mu_a}
        )
    nc = _get_nc()
    kw = {}
    if _trace:
        import shutil
        import tempfile

        shutil.rmtree("/tmp/vqc_prof", ignore_errors=True)
        os.makedirs("/tmp/vqc_prof", exist_ok=True)
        kw["tmpdir"] = tempfile.mkdtemp(dir="/tmp/vqc_prof")
    res = run_bass_kernel_spmd(nc, in_maps, list(range(N_CORES)), trace=_trace, **kw)
    outs = []
    for i in range(N_CORES):
        raw = res.results[i]["out"]  # (4, 32): [k, q*16 + g]
        e = np.empty((BPC, 2), dtype=np.float32)
        for g in range(16):
            for k in range(4):
                for q in range(2):
                    e[g * 4 + k, q] = raw[k, q * 16 + g]
        outs.append(e)
    full = np.concatenate(outs, axis=0).astype(np.float32)
    if _trace:
        _nc_cache["last_exec_ns"] = res.exec_time_ns
        _nc_cache["last_results"] = res
    return full


# revision 37
# speedup vs baseline: 1.0359x; 1.0359x over previous
"""Trainium2 Bass kernel for nn_ANO_VQC_Model (14-qubit VQC, batch 512).

Math: the circuit state, viewed as a 128x128 matrix M (rows = qubits 0-6,
cols = qubits 7-13), starts as a real rank-1 outer product u v^T and each
entangling layer k acts as M' = A_k CNOT67(M) B_k^T with A_k, B_k pure
orthogonal local operators and CNOT67(M) = E0 M + E1 M F (E0/E1 = projectors
on qubit 6 = row LSB, F = X on qubit 7 = column-half swap).  So the state
stays factored L R^T with L <- A[E0 L | E1 L], R <- B[R | F R]; rank doubles
per layer.

Only 5 layers are simulated (rank 32).  The 6th layer is folded into the
measurement: with M6 = A6 N B6^T, N = CNOT67(M5), orthogonality of B6 gives
    e_q = <T_dq, S> + <T_oq, S'>,
    T_dq = L^T Hd_q L, T_oq = L^T Ho_q L, S = R^T R, S' = R^T F R,
where Hd_q/Ho_q are the qubit-6 block-diagonal/off-diagonal parts of
A6^T Re(H_q) A6.  A host-side orthogonal rotation Q diagonalizes Hd_0
(folded into the stage-2 A matrices), so T_d0 needs only a per-partition
scale of L.

Device schedule (v3): stage1 = layers 0-1 (b-major psum writes so stage2
streams contiguously), stage2 = layers 2-4 (contiguous 256-col rhs), K =
the three measurement matrix products (512-col psum chunks), per-sample
Grams (4-sample PE column tiling, ~34ns/mm issue rate), psum-direct
multiplies + c-accumulated selection matmuls + PSUM->SBUF reduces.  All
matmuls fp16 (fp8 was measured: each state quantization costs ~2.5e-2 rel
err, over budget; stride-0 DoubleRow gives no speedup).  Weights arrive as
packed tensors (1-2KB DMA rows) and the warmup is reduced to two bridge
matmuls so real work starts immediately.

Sharding: pure data parallel, 64 batch elements per core on 8 cores.
"""

import os
import sys

import numpy as np

for _p in ("/opt/trn_rl_repo", "/root/.axon_site/_ro/trn_rl_repo"):
    if os.path.isdir(_p) and _p not in sys.path:
        sys.path.append(_p)

import concourse.bass as bass
import concourse.mybir as mybir
import concourse.tile as tile
from concourse import bacc
from concourse.bass_utils import run_bass_kernel_spmd


def _ensure_ntff_hook():
    """bass_utils imports antenv.axon_hooks when tracing; some images lack
    it.  Provide a shim (and register the ctypes NTFF hook when possible)."""
    try:
        import antenv.axon_hooks  # noqa: F401

        return
    except ImportError:
        pass
    try:
        import types

        import antenv

        mod = types.ModuleType("antenv.axon_hooks")
        holder = {}
        mod.set_axon_ntff_profile_hook = lambda h: holder.__setitem__("h", h)
        mod.get_axon_ntff_profile_hook = lambda: holder.get("h")
        sys.modules["antenv.axon_hooks"] = mod
        antenv.axon_hooks = mod
        try:
            from trn_agent_boot.trn_boot import _ntff_profile_via_ctypes

            hook = _ntff_profile_via_ctypes("/opt/axon/libaxon_pjrt.so")
            if hook is not None:
                mod.set_axon_ntff_profile_hook(hook)
        except Exception:
            pass
    except Exception:
        pass


_ensure_ntff_hook()

N_CORES = 8
BATCH = 512
BPC = BATCH // N_CORES  # 64
DEPTH = 6
DA = 128
DB = 128

F32 = mybir.dt.float32
F16 = mybir.dt.float16
F8 = mybir.dt.float8e4
DRMODE = mybir.MatmulPerfMode.DoubleRow

USE_FP8 = os.environ.get("VQC_FP8", "0") == "1"
MM8 = F8 if USE_FP8 else F16

_nc_cache = {}


# ----------------------------------------------------------------------------
# Host-side preprocessing (input-dependent constant folding)
# ----------------------------------------------------------------------------

def _ry(t):
    c, s = np.cos(t / 2), np.sin(t / 2)
    return np.array([[c, -s], [s, c]], dtype=np.float64)


_CNOT = np.array(
    [[1, 0, 0, 0], [0, 1, 0, 0], [0, 0, 0, 1], [0, 0, 1, 0]], dtype=np.float64
)


def _kron_list(ms):
    out = ms[0]
    for m in ms[1:]:
        out = np.kron(out, m)
    return out


def _cnot_on(n, ctrl):
    mats, q = [], 0
    while q < n:
        if q == ctrl:
            mats.append(_CNOT)
            q += 2
        else:
            mats.append(np.eye(2))
            q += 1
    return _kron_list(mats)


def _layer_ops(theta_k):
    """Pure-orthogonal (A, B) for one layer; CNOT67 handled separately."""
    C_evenA = _cnot_on(7, 0) @ _cnot_on(7, 2) @ _cnot_on(7, 4)
    C_oddA = _cnot_on(7, 1) @ _cnot_on(7, 3) @ _cnot_on(7, 5)
    R_A = _kron_list([_ry(theta_k[w]) for w in range(7)])
    C_evenB = _cnot_on(7, 1) @ _cnot_on(7, 3) @ _cnot_on(7, 5)
    C_oddB = _cnot_on(7, 0) @ _cnot_on(7, 2) @ _cnot_on(7, 4)
    R_B = _kron_list([_ry(theta_k[7 + w]) for w in range(7)])
    return R_A @ C_oddA @ C_evenA, R_B @ C_oddB @ C_evenB


def _measure_mats(Ain, Din):
    """G_q = Re(H_q) expanded on the 128-dim row space, q = 0, 1."""
    NLOC = 8
    r, c = np.tril_indices(NLOC, -1)
    Gs = []
    for q in range(2):
        tri = np.zeros((NLOC, NLOC))
        tri[r, c] = Ain[q]
        h = tri + np.diag(np.concatenate([Din[q][1:], [0.0]]))
        Hr = h + h.T
        if q == 0:
            Gs.append(np.kron(Hr, np.eye(16)))
        else:
            Gs.append(np.kron(np.kron(np.eye(2), Hr), np.eye(8)))
    return Gs


def _host_prep(X, theta, Ain, Bin, Din):
    X = np.asarray(X, dtype=np.float64)
    theta = np.asarray(theta, dtype=np.float64)
    nb = X.shape[0]
    c, s = np.cos(X / 2), np.sin(X / 2)
    v0 = (c - s) / np.sqrt(2.0)
    v1 = (c + s) / np.sqrt(2.0)

    def kron_side(ws):
        out = np.ones((nb, 1))
        for w in ws:
            pair = np.stack([v0[:, w], v1[:, w]], axis=1)
            out = (out[:, :, None] * pair[:, None, :]).reshape(nb, -1)
        return out

    U = kron_side(range(7))       # (B, 128), qubit 0 = MSB ... qubit 6 = LSB
    V = kron_side(range(7, 14))   # (B, 128), qubit 7 = MSB

    As, Bs = zip(*[_layer_ops(theta[k]) for k in range(DEPTH)])
    rows = np.arange(DA)
    e0 = (rows % 2 == 0).astype(np.float64)
    E = [np.diag(e0), np.diag(1.0 - e0)]
    F = np.zeros((DB, DB))
    F[:64, 64:] = np.eye(64)
    F[64:, :64] = np.eye(64)

    # measurement: fold layer 6, rotate rows by Q diagonalizing Hd_0
    G0, G1 = _measure_mats(Ain, Din)
    A6 = As[5]
    Hds, Hos = [], []
    for G in (G0, G1):
        Ht = A6.T @ G @ A6
        Hds.append(E[0] @ Ht @ E[0] + E[1] @ Ht @ E[1])
        Hos.append(E[0] @ Ht @ E[1] + E[1] @ Ht @ E[0])
    mu, Q = np.linalg.eigh(Hds[0])
    hk = np.stack(
        [Q.T @ Hos[0] @ Q, Q.T @ Hds[1] @ Q, Q.T @ Hos[1] @ Q], axis=0
    )  # (3, 128, 128) symmetric

    # stage 1: layers 0-1, 4 terms; c2 = p1*2 + p0
    F1A = np.empty((4, DA, DA))
    F1B = np.empty((4, DB, DB))
    for cw in range(4):
        p0, p1 = cw & 1, (cw >> 1) & 1
        F1A[cw] = As[1] @ E[p1] @ As[0] @ E[p0]
        F1B[cw] = (
            Bs[1] @ np.linalg.matrix_power(F, p1)
            @ Bs[0] @ np.linalg.matrix_power(F, p0)
        )
    # stage 2: layers 2-4, 8 terms; a = p4*4 + p3*2 + p2; Q^T folded into A
    F2A = np.empty((8, DA, DA))
    F2B = np.empty((8, DB, DB))
    for aw_ in range(8):
        p2, p3, p4 = aw_ & 1, (aw_ >> 1) & 1, (aw_ >> 2) & 1
        F2A[aw_] = Q.T @ As[4] @ E[p4] @ As[3] @ E[p3] @ As[2] @ E[p2]
        F2B[aw_] = (
            Bs[4] @ np.linalg.matrix_power(F, p4)
            @ Bs[3] @ np.linalg.matrix_power(F, p3)
            @ Bs[2] @ np.linalg.matrix_power(F, p2)
        )

    # lhsT packs.  B-side F-pairing: F1B[(p1,1)] = F1B[(p1,0)] @ F, so store
    # only p0=0 terms and stream [vt | F vt]; the F-premultiplied copies
    # (F @ F1B, F @ F2B) produce the partition-swapped states F R directly,
    # so no SBUF-SBUF swap DMAs are needed anywhere.
    wa1 = np.concatenate([F1A[cw].T for cw in range(4)], axis=1)  # (128, 512)
    wa2 = np.concatenate([F2A[aw_].T for aw_ in range(8)], axis=1)  # (128,1024)
    wb1c = np.concatenate(
        [F1B[p1 * 2].T for p1 in range(2)]
        + [(F @ F1B[p1 * 2]).T for p1 in range(2)],
        axis=1,
    )  # (128, 512): [normal(2) | F-pre(2)]; term (p1,p0) uses rhs half p0
    wb2c = np.concatenate(
        [F2B[(p4 * 2 + p3) * 2].T for p4 in range(2) for p3 in range(2)],
        axis=1,
    )  # (128, 512), ab = p4*2+p3
    wb2f = np.concatenate(
        [(F @ F2B[(p4 * 2 + p3) * 2]).T for p4 in range(2) for p3 in range(2)],
        axis=1,
    )  # (128, 512)
    whk = np.concatenate([hk[i] for i in range(3)], axis=1)  # (128, 384)
    return U, V, wa1, wb1c, wa2, wb2c, wb2f, whk, mu


# ----------------------------------------------------------------------------
# Device kernel
# ----------------------------------------------------------------------------

def _dr(ap, n):
    """Duplicate the contraction as a stride-0 second k-subtile (DoubleRow)."""
    return ap.unsqueeze(1).broadcast_to((128, 2) + tuple(n))


def _build_nc():
    nc = bacc.Bacc("TRN2", target_bir_lowering=False, debug=False)

    # pk1 = [ut | wa1c], pk2 = [vt | F vt | wb1c]; parity-compressed A-side
    # weights, F-paired B-side weights (half the DMA bytes each)
    pk1_d = nc.declare_dram_parameter("pk1", [DA, 576], F16, isOutput=False)
    pk2_d = nc.declare_dram_parameter("pk2", [DB, 640], F16, isOutput=False)
    wa2_d = nc.declare_dram_parameter("wa2c", [DA, 1024], F16, isOutput=False)
    wb2_d = nc.declare_dram_parameter("wb2c", [DB, 512], F16, isOutput=False)
    wb2f_d = nc.declare_dram_parameter("wb2f", [DB, 512], F16, isOutput=False)
    whk_d = nc.declare_dram_parameter("whk", [DA, 384], F16, isOutput=False)
    mu_d = nc.declare_dram_parameter("mu", [DA, 1], F32, isOutput=False)
    out_d = nc.declare_dram_parameter("out", [4, 32], F32, isOutput=True)

    with tile.TileContext(nc) as tc:
        with (
            tc.tile_pool(name="w", bufs=1) as wpool,
            tc.tile_pool(name="state", bufs=1) as spool,
            tc.tile_pool(name="ps", bufs=3, space="PSUM") as pbig,   # 2 banks each
            tc.tile_pool(name="ps2", bufs=2, space="PSUM") as psmall,  # 1 bank each
        ):
            pk1 = wpool.tile([DA, 576], F16, tag="pk1")
            pk2 = wpool.tile([DB, 640], F16, tag="pk2")
            wa2 = wpool.tile([DA, 1024], F16, tag="wa2c")
            wb2 = wpool.tile([DB, 512], F16, tag="wb2c")
            wb2f = wpool.tile([DB, 512], F16, tag="wb2f")
            hkw = wpool.tile([DA, 384], F16, tag="hkw")
            muT = wpool.tile([DA, 1], F32, tag="mu")
            sel = wpool.tile([128, 4], F16, tag="sel")
            warm = wpool.tile([128, 512], F16, tag="warm")

            ut = pk1[:, 0:64]
            wa1 = pk1[:, 64:576]
            vtp = pk2[:, 0:128]
            wb1 = pk2[:, 128:640]

            # input DMAs over the three DMA-capable queues (~48, ~48,
            # ~32 GB/s observed), earliest-needed first per queue
            nc.sync.dma_start(out=pk1[:], in_=pk1_d[:, :])
            nc.scalar.dma_start(out=wa2[:], in_=wa2_d[:, :])
            nc.gpsimd.dma_start(out=pk2[:], in_=pk2_d[:, :])
            nc.sync.dma_start(out=hkw[:], in_=whk_d[:, :])
            nc.scalar.dma_start(out=wb2f[:], in_=wb2f_d[:, :])
            nc.gpsimd.dma_start(out=wb2[:], in_=wb2_d[:, :])
            nc.gpsimd.dma_start(out=muT[:], in_=mu_d[:, :])

            nc.vector.memset(warm[:], 0.125)
            nc.vector.memset(sel[:], 0.0)
            for m in range(4):
                nc.vector.memset(sel[32 * m:32 * m + 32, m:m + 1], 1.0)

            # one warmup matmul bridges the PE until the first input DMA lands
            wps = pbig.tile([128, 1024], F32, tag="big")
            nc.tensor.matmul(
                wps[:, 0:512], warm[:, 0:128], warm[:], start=True, stop=True
            )
            nc.tensor.matmul(
                wps[:, 512:1024], warm[:, 0:128], warm[:], start=True, stop=True
            )

            L3 = spool.tile([DA, 256], MM8, tag="L3")      # cols (b, c)
            R3p = spool.tile([DB, 512], MM8, tag="R3p")    # [R3 | F R3]
            Lb = spool.tile([DA, 2048], MM8, tag="Lb")     # cols (b, a, c)
            RF = spool.tile([DB, 4096], MM8, tag="RF")     # 0:2048 R, 2048: FR
            Pb = spool.tile([DA, 8192], MM8, tag="Pb")     # cols (b, s4, i32)
            SS = spool.tile([128, 1024], F16, tag="SS")
            tb = spool.tile([128, 2048], F16, tag="tb")
            esb = spool.tile([4, 32], F32, tag="esb")

            # ---- stage 1: 4 terms per side, b-major psum writes -------------
            # A side: parity-compressed weights, 64-deep contraction on the
            # partition half selected by p0.  B side: F-paired weights with
            # the [vt | F vt] rhs pack.  Separate psum tiles per side so the
            # L3 evict does not wait on the B-side matmuls.
            s1t = psmall.tile([128, 256], F32, tag="small")
            s1u = psmall.tile([128, 512], F32, tag="small")
            s1a = s1t[:].rearrange("p (b c) -> p b c", c=4)
            s1b = s1u[:].rearrange("p (f b c) -> p f b c", f=2, c=4)
            for cw in range(4):
                nc.tensor.matmul(
                    s1a[:, :, cw],
                    wa1[:, cw * 128:(cw + 1) * 128], ut,
                    start=True, stop=True, skip_group_check=True,
                )
            nc.vector.tensor_copy(L3[:], s1t[:])

            # ---- stage 2 A first (only needs L3/wa2): keeps the in-order PE
            # queue from stalling on the later pk2 DMA ----------------------
            Lbv = Lb[:].rearrange("p (b a c) -> p a b c", a=8, c=4)
            Rbv = RF[:, 0:2048].rearrange("p (b a c) -> p a b c", a=8, c=4)
            evict_engines = [nc.vector, nc.scalar]

            def coarse_evict(eng, dst, srcv):
                if eng is nc.scalar:
                    eng.copy(out=dst, in_=srcv)
                else:
                    eng.tensor_copy(dst, srcv)

            # A side: 8 terms, contiguous 256-col rhs
            for half in range(2):
                s2 = pbig.tile([128, 1024], F32, tag="big")
                for i in range(4):
                    a = 4 * half + i
                    nc.tensor.matmul(
                        s2[:, i * 256:(i + 1) * 256],
                        wa2[:, a * 128:(a + 1) * 128], L3[:],
                        start=True, stop=True,
                    )
                s2v = s2[:].rearrange("p (i b c) -> p b i c", i=4, c=4)
                coarse_evict(
                    evict_engines[half % 2],
                    Lbv[:, 4 * half:4 * half + 4].rearrange(
                        "p a b c -> p b a c"
                    ),
                    s2v,
                )

            Pv0 = Pb[:].rearrange("p (b s i) -> p s b i", s=4, i=32)

            # ---- stage 1 B (waits on the pk2 DMA) ---------------------------
            for fw in range(2):
                for cw in range(4):
                    p0, p1 = cw & 1, (cw >> 1) & 1
                    nc.tensor.matmul(
                        s1b[:, fw, :, cw],
                        wb1[:, (fw * 2 + p1) * 128:(fw * 2 + p1 + 1) * 128],
                        vtp[:, p0 * 64:(p0 + 1) * 64],
                        start=True, stop=True, skip_group_check=True,
                    )
            nc.scalar.copy(out=R3p[:], in_=s1u[:])
            # B side: ab = p4*2 + p3; one matmul streams [R3 | F R3] and
            # produces both p2 terms (a = 2*ab + p2).  The wb2f pass emits
            # the partition-swapped F R states straight into RF[:, 2048:].
            FRbv = RF[:, 2048:4096].rearrange("p (b a c) -> p a b c", a=8, c=4)
            for fw, (wtile, dstv) in enumerate(((wb2, Rbv), (wb2f, FRbv))):
                for half in range(2):
                    s2 = pbig.tile([128, 1024], F32, tag="big")
                    for m in range(2):
                        ab = 2 * half + m
                        nc.tensor.matmul(
                            s2[:, m * 512:(m + 1) * 512],
                            wtile[:, ab * 128:(ab + 1) * 128],
                            R3p[:],
                            start=True, stop=True,
                        )
                    s2v = s2[:].rearrange(
                        "p (m f b c) -> p b m f c", m=2, f=2, c=4
                    )
                    coarse_evict(
                        evict_engines[(half + fw + 1) % 2],
                        dstv[:, 4 * half:4 * half + 4].rearrange(
                            "p (m f) b c -> p b m f c", m=2
                        ),
                        s2v,
                    )

            # ---- K matmuls: 256-col chunks into 1-bank psum tiles -----------
            # rhs chunk = Lb[b-half, a-pair j] -> [p, 32, 8] contiguous runs
            Lbk = Lb[:].rearrange("p (b i) -> p b i", i=32)
            Pbv = Pb[:].rearrange("p (b s i) -> p b s i", s=4, i=32)

            def k_mat(im):
                for bh in range(2):
                    kp = pbig.tile([128, 1024], F32, tag="big")
                    for j in range(4):
                        rhs = Lbk[:, 32 * bh:32 * bh + 32, 8 * j:8 * j + 8]
                        lhs = hkw[:, im * 128:(im + 1) * 128]
                        nc.tensor.matmul(
                            kp[:, j * 256:(j + 1) * 256], lhs, rhs,
                            start=True, stop=True,
                        )
                    # evict: psum (j, b32, i8) -> Pb[b, im+1, (j,i8)]
                    kv = kp[:].rearrange("p (j b i) -> p b j i", j=4, i=8)
                    dst = Pbv[:, 32 * bh:32 * bh + 32, im + 1].rearrange(
                        "p b (j i) -> p b j i", j=4
                    )
                    eng = evict_engines[(im + bh) % 2]
                    if eng is nc.scalar:
                        eng.copy(out=dst, in_=kv)
                    else:
                        eng.tensor_copy(dst, kv)

            # ---- per-sample S-Grams: rhs = [R_b | FR_b] ---------------------
            RFs = RF[:].rearrange("p (h b i) -> p b h i", h=2, i=32)

            def gram_s(g2):
                sp = psmall.tile([128, 512], F32, tag="small")
                for g in range(8 * g2, 8 * g2 + 8):
                    for k in range(4):
                        b = g * 4 + k
                        nc.tensor.matmul(
                            sp[32 * k:32 * k + 32,
                               (g - 8 * g2) * 64:(g - 8 * g2 + 1) * 64],
                            RF[:, b * 32:(b + 1) * 32],
                            RFs[:, b],
                            start=True, stop=True, tile_position=(0, 32 * k),
                        )
                nc.scalar.copy(out=SS[:, g2 * 512:(g2 + 1) * 512], in_=sp[:])

            # interleave K-matrix matmuls with S-Grams to keep the PE dense;
            # mu*L (ACT, per-partition scale) sits after the early k-evicts
            k_mat(0)
            k_mat(1)
            nc.vector.tensor_scalar_mul(
                Pv0[:, 0], Lb[:].rearrange("p (b i) -> p b i", i=32), muT[:]
            )
            gram_s(0)
            k_mat(2)
            gram_s(1)

            # ---- T-Grams + psum-direct multiply -----------------------------
            for g2 in range(2):
                tp = pbig.tile([128, 1024], F32, tag="big")
                for g in range(8 * g2, 8 * g2 + 8):
                    for k in range(4):
                        b = g * 4 + k
                        nc.tensor.matmul(
                            tp[32 * k:32 * k + 32,
                               (g - 8 * g2) * 128:(g - 8 * g2 + 1) * 128],
                            Lb[:, b * 32:(b + 1) * 32],
                            Pb[:, b * 128:(b + 1) * 128],
                            start=True, stop=True, tile_position=(0, 32 * k),
                        )
                ssv = SS[:, g2 * 512:(g2 + 1) * 512].rearrange(
                    "p (g j) -> p g j", g=8
                ).unsqueeze(2).broadcast_to((128, 8, 2, 64))
                tbv = tb[:, g2 * 1024:(g2 + 1) * 1024].rearrange(
                    "p (g q j) -> p g q j", g=8, q=2, j=64
                )
                tpv = tp[:].rearrange("p (g q j) -> p g q j", g=8, q=2, j=64)
                # psum-direct multiply on DVE (two chunks for pipelining)
                nc.vector.tensor_mul(tbv[:, 0:4], tpv[:, 0:4], ssv[:, 0:4])
                nc.vector.tensor_mul(tbv[:, 4:8], tpv[:, 4:8], ssv[:, 4:8])

            # ---- partition reduce: c-accumulated selection matmuls ----------
            zp0 = psmall.tile([4, 256], F32, tag="small")
            zp1 = psmall.tile([4, 256], F32, tag="small")
            zpq = [zp0, zp1]
            for g2 in range(2):
                tbq = tb[:, g2 * 1024:(g2 + 1) * 1024].rearrange(
                    "p (g q c j) -> p q c g j", g=8, q=2, c=4, j=16
                )
                for q in range(2):
                    for c in range(4):
                        nc.tensor.matmul(
                            zpq[q][:, g2 * 128:(g2 + 1) * 128],
                            sel[:], tbq[:, q, c],
                            start=(c == 0), stop=(c == 3),
                        )
            for g2 in range(2):
                for q in range(2):
                    nc.vector.reduce_sum(
                        out=esb[:, q * 16 + g2 * 8:q * 16 + g2 * 8 + 8],
                        in_=zpq[q][:, g2 * 128:(g2 + 1) * 128].rearrange(
                            "p (g j) -> p g j", j=16
                        ),
                        axis=mybir.AxisListType.X,
                    )
            nc.sync.dma_start(out=out_d[:, :], in_=esb[:])

    nc.compile()
    return nc


def _get_nc():
    if "nc" not in _nc_cache:
        _nc_cache["nc"] = _build_nc()
    return _nc_cache["nc"]


# ----------------------------------------------------------------------------
# Entry point
# ----------------------------------------------------------------------------

def _to_mm8(x):
    if USE_FP8:
        import ml_dtypes

        return np.ascontiguousarray(x, dtype=ml_dtypes.float8_e4m3fn)
    return np.ascontiguousarray(x, dtype=np.float16)


def kernel(X, theta, A, B, D, _trace=False):
    U, V, wa1, wb1c, wa2, wb2c, wb2f, whk, mu = _host_prep(X, theta, A, B, D)
    wa2_a = np.ascontiguousarray(wa2, dtype=np.float16)
    wb2_a = np.ascontiguousarray(wb2c, dtype=np.float16)
    wb2f_a = np.ascontiguousarray(wb2f, dtype=np.float16)
    whk_a = np.ascontiguousarray(whk, dtype=np.float16)
    mu_a = np.ascontiguousarray(mu.reshape(DA, 1), dtype=np.float32)
    in_maps = []
    for i in range(N_CORES):
        sl = slice(i * BPC, (i + 1) * BPC)
        vt = V[sl].T
        fvt = np.concatenate([vt[64:128], vt[0:64]], axis=0)
        pk1 = np.ascontiguousarray(
            np.concatenate([U[sl].T, wa1], axis=1), dtype=np.float16
        )
        pk2 = np.ascontiguousarray(
            np.concatenate([vt, fvt, wb1c], axis=1), dtype=np.float16
        )
        in_maps.append(
            {"pk1": pk1, "pk2": pk2, "wa2c": wa2_a, "wb2c": wb2_a,
             "wb2f": wb2f_a, "whk": whk_a, "mu": mu_a}
        )
    nc = _get_nc()
    kw = {}
    if _trace:
        import shutil
        import tempfile

        shutil.rmtree("/tmp/vqc_prof", ignore_errors=True)
        os.makedirs("/tmp/vqc_prof", exist_ok=True)
        kw["tmpdir"] = tempfile.mkdtemp(dir="/tmp/vqc_prof")
    res = run_bass_kernel_spmd(nc, in_maps, list(range(N_CORES)), trace=_trace, **kw)
    outs = []
    for i in range(N_CORES):
        raw = res.results[i]["out"]  # (4, 32): [k, q*16 + g]
        e = np.empty((BPC, 2), dtype=np.float32)
        for g in range(16):
            for k in range(4):
                for q in range(2):
                    e[g * 4 + k, q] = raw[k, q * 16 + g]
        outs.append(e)
    full = np.concatenate(outs, axis=0).astype(np.float32)
    if _trace:
        _nc_cache["last_exec_ns"] = res.exec_time_ns
        _nc_cache["last_results"] = res
    return full
